# revision 13
# baseline (speedup 1.0000x reference)
"""Trainium2 Bass kernel for nn_DynamicGraphLearner (gnn_message_passing).

Computes, for an N-node graph:
  h = emb @ W.T  (heads x hd), attention scores s_dst/s_src per head,
  scores[i,j] = mean_h leaky_relu(s_dst[i,h] + s_src[j,h]) masked by
  (active & sector_mask), plus a scattered correlation prior, then per-row
  top-15 + softmax -> (edge_index[2, N*15], edge_weight[N*15]).

Sharding: destination-node axis (rows) split across 8 NeuronCores, 512 rows
per core.  Embeddings/W/att replicated; each core computes the full
projection (cheap) and its own 512x4096 score block, local top-k + softmax.

Algebra used on-device (exact in f32 up to fp reassociation):
  leaky_relu(x) = 0.2*x + 0.8*relu(x)
  scores = sum_h relu(0.2*(s_dst_h + s_src_h))            [4 relu terms]
         + 0.05*sd_sum[i] + 0.05*ss_sum[j]                [linear term]
         + combo[i,j]
  combo[i,j] = where(mask, lambda*corr, -1e30)  (host-merged input data)

The projection is done on-device: emb is transposed via PE transposes, then
sT[g, n] = (W.T @ Asp).T-style fused matmul with a host-built sparse
placement matrix Asp[256, 16] (pure re-layout of the `att` input):
  g=0..3 : 0.2*s_dst_h     g=4..7 : 0.2*s_src_h
  g=8    : 0.05*sd_sum     g=9    : 0.05*ss_sum
"""

import os
import numpy as np
from contextlib import ExitStack

N, H, HEADS, HD = 4096, 256, 4, 64
TOP_K = 15
NEG_SLOPE = 0.2
NCORES = 8
RPC = N // NCORES          # rows per core = 512
RT = RPC // 128            # row-tiles per core = 4
MASK_NEG = -1.0e30
REPLACED = -3.0e38

_PROGRAM_CACHE = {}


def _build_program():
    """Build + compile the Bass/Tile program (shared SPMD across 8 cores)."""
    if "nc" in _PROGRAM_CACHE:
        return _PROGRAM_CACHE["nc"]

    import concourse.bacc as bacc
    import concourse.bass as bass
    import concourse.mybir as mybir
    import concourse.tile as tile
    from concourse.masks import make_identity

    dt = mybir.dt
    f32 = dt.float32
    Alu = mybir.AluOpType
    Act = mybir.ActivationFunctionType
    X = mybir.AxisListType.X

    nc = bacc.Bacc(
        "TRN2", target_bir_lowering=False, debug=False, num_devices=NCORES
    )

    emb = nc.dram_tensor("emb", [N, H], f32, kind="ExternalInput").ap()
    embr = nc.dram_tensor("embr", [RPC, H], f32, kind="ExternalInput").ap()
    asp = nc.dram_tensor("asp", [H, 16], f32, kind="ExternalInput").ap()
    wmat = nc.dram_tensor("wmat", [H, H], f32, kind="ExternalInput").ap()
    combo = nc.dram_tensor("combo", [RPC, N], f32, kind="ExternalInput").ap()
    oidx = nc.dram_tensor("oidx", [RPC, TOP_K], dt.int32, kind="ExternalOutput").ap()
    owgt = nc.dram_tensor("owgt", [RPC, TOP_K], f32, kind="ExternalOutput").ap()

    with tile.TileContext(nc) as tc:
        with ExitStack() as ctx:
            const = ctx.enter_context(tc.tile_pool(name="const", bufs=1))
            psum = ctx.enter_context(tc.tile_pool(name="psum", bufs=1, space="PSUM"))
            stage = ctx.enter_context(tc.tile_pool(name="stage", bufs=3))
            cmbp = ctx.enter_context(tc.tile_pool(name="cmbp", bufs=2))
            wk = ctx.enter_context(tc.tile_pool(name="wk", bufs=4))
            bcp = ctx.enter_context(tc.tile_pool(name="bcp", bufs=1))
            small = ctx.enter_context(tc.tile_pool(name="small", bufs=4))

            idt = const.tile([128, 128], f32, tag="idt")
            make_identity(nc, idt[:])

            # --- load W [r, c] (lhsT for the V matmul) and Asp [r, 16] ---
            w_t = []
            asp_t = []
            for rhalf in range(2):
                wt = const.tile([128, H], f32, tag=f"w{rhalf}")
                nc.sync.dma_start(out=wt[:], in_=wmat[rhalf * 128:(rhalf + 1) * 128, :])
                w_t.append(wt)
                at = const.tile([128, 16], f32, tag=f"asp{rhalf}")
                nc.sync.dma_start(out=at[:], in_=asp[rhalf * 128:(rhalf + 1) * 128, :])
                asp_t.append(at)

            # --- V'[c, g] = sum_r W[r, c] * Asp[r, g]  (two c-halves) ---
            v_t = []
            for chalf in range(2):
                vp = psum.tile([128, 16], f32, tag="vps", bufs=1)
                for rhalf in range(2):
                    nc.tensor.matmul(
                        vp[:],
                        w_t[rhalf][:, chalf * 128:(chalf + 1) * 128],
                        asp_t[rhalf][:],
                        start=(rhalf == 0),
                        stop=(rhalf == 1),
                    )
                vs = const.tile([128, 16], f32, tag=f"v{chalf}")
                nc.vector.tensor_copy(vs[:], vp[:])
                v_t.append(vs)

            # --- transpose emb -> embT [c, n] (two c-half tiles) ---
            embT = [wk.tile([128, N], f32, tag="wk", name=f"embT{i}") for i in range(2)]
            for nt in range(N // 128):
                et = stage.tile([128, H], f32, tag="emb_stage")
                nc.sync.dma_start(out=et[:], in_=emb[nt * 128:(nt + 1) * 128, :])
                for chalf in range(2):
                    # 128x128 PE transpose; batch 4 per PSUM bank-tile? keep 1:1 simple
                    tp = psum.tile([128, 128], f32, tag="tps", bufs=3)
                    nc.tensor.transpose(
                        tp[:], et[:, chalf * 128:(chalf + 1) * 128], idt[:]
                    )
                    nc.scalar.copy(embT[chalf][:, nt * 128:(nt + 1) * 128], tp[:])

            # --- sT[g, n] = sum_c V'[c, g] * embT[c, n] ---
            sT = wk.tile([16, N], f32, tag="wk", name="sT")
            for chunk in range(N // 512):
                sp = psum.tile([16, 512], f32, tag="sps", bufs=2)
                for chalf in range(2):
                    nc.tensor.matmul(
                        sp[:],
                        v_t[chalf][:],
                        embT[chalf][:, chunk * 512:(chunk + 1) * 512],
                        start=(chalf == 0),
                        stop=(chalf == 1),
                    )
                nc.scalar.copy(sT[:, chunk * 512:(chunk + 1) * 512], sp[:])

            # --- per-row-tile dst biases from this core's OWN emb rows ---
            # bias[n, g] = sum_c embr[n, c] * V'[c, g]; lhsT must be [c, n],
            # so transpose each embr row-tile first (SPMD: embr is the only
            # per-core row-identity input besides combo).
            biases = []
            for t in range(RT):
                ert = stage.tile([128, H], f32, tag="emb_stage", name=f"ert{t}")
                nc.sync.dma_start(out=ert[:], in_=embr[t * 128:(t + 1) * 128, :])
                ett = []
                for chalf in range(2):
                    tp2 = psum.tile([128, 128], f32, tag="tps", bufs=3)
                    nc.tensor.transpose(
                        tp2[:], ert[:, chalf * 128:(chalf + 1) * 128], idt[:]
                    )
                    es = stage.tile([128, 128], f32, tag="ett", bufs=2,
                                    name=f"ett{t}_{chalf}")
                    nc.scalar.copy(es[:], tp2[:])
                    ett.append(es)
                bp = psum.tile([128, 16], f32, tag="bps", bufs=1)
                for chalf in range(2):
                    nc.tensor.matmul(
                        bp[:], ett[chalf][:], v_t[chalf][:],
                        start=(chalf == 0), stop=(chalf == 1),
                    )
                bt = const.tile([128, 16], f32, tag=f"bias{t}")
                nc.vector.tensor_copy(bt[:], bp[:])
                biases.append(bt)

            # --- broadcast source-rows (heads + ss_lin) across partitions ---
            # partition_broadcast needs its source on partition 0: bounce each
            # sT row there via a small SBUF->SBUF DMA first.
            bc = []
            for h in range(HEADS):
                rb = cmbp.tile([1, N], f32, tag="cmb", name=f"rb{h}")
                nc.sync.dma_start(out=rb[:], in_=sT[4 + h:5 + h, :])
                b = bcp.tile([128, N], f32, tag=f"bc{h}")
                nc.gpsimd.partition_broadcast(b[:], rb[:], channels=128)
                bc.append(b)
            rbl = cmbp.tile([1, N], f32, tag="cmb", name="rbl")
            nc.sync.dma_start(out=rbl[:], in_=sT[9:10, :])
            bc_lin = bcp.tile([128, N], f32, tag="bc_lin")
            nc.gpsimd.partition_broadcast(bc_lin[:], rbl[:], channels=128)

            # --- main loop over row tiles ---
            for t in range(RT):
                bt = biases[t]
                r0 = wk.tile([128, N], f32, tag="wk", name=f"r0_{t}")
                nc.vector.tensor_scalar(
                    r0[:], bc[0][:], bt[:, 0:1], 0.0, op0=Alu.add, op1=Alu.max
                )
                r1 = wk.tile([128, N], f32, tag="wk", name=f"r1_{t}")
                nc.vector.tensor_scalar(
                    r1[:], bc[1][:], bt[:, 1:2], 0.0, op0=Alu.add, op1=Alu.max
                )
                ta = wk.tile([128, N], f32, tag="wk", name=f"ta_{t}")
                nc.vector.tensor_add(ta[:], r0[:], r1[:])

                r2 = wk.tile([128, N], f32, tag="wk", name=f"r2_{t}")
                nc.scalar.activation(r2[:], bc[2][:], Act.Relu, bias=bt[:, 2:3], scale=1.0)
                r3 = wk.tile([128, N], f32, tag="wk", name=f"r3_{t}")
                nc.scalar.activation(r3[:], bc[3][:], Act.Relu, bias=bt[:, 3:4], scale=1.0)
                tb = wk.tile([128, N], f32, tag="wk", name=f"tb_{t}")
                # tb = (r2 + sd_lin) + r3
                nc.vector.scalar_tensor_tensor(
                    tb[:], r2[:], bt[:, 8:9], r3[:], op0=Alu.add, op1=Alu.add
                )
                # combo rows + ss_lin broadcast, merged on gpsimd (accumulating
                # DMA is broken on this runtime path — do it with ALU ops)
                cmb = cmbp.tile([128, N], f32, tag="cmb", name=f"cmb_{t}")
                nc.sync.dma_start(out=cmb[:], in_=combo[t * 128:(t + 1) * 128, :])
                nc.gpsimd.tensor_add(cmb[:], cmb[:], bc_lin[:])

                tcc = wk.tile([128, N], f32, tag="wk", name=f"tc_{t}")
                nc.vector.tensor_add(tcc[:], ta[:], tb[:])
                nc.gpsimd.tensor_add(tcc[:], tcc[:], cmb[:])

                # --- top-16 via two rounds of max8 (match_replace in place) ---
                vals = small.tile([128, 16], f32, tag="vals")
                idx = small.tile([128, 16], dt.uint32, tag="idx")
                nc.vector.max(vals[:, 0:8], tcc[:])
                nc.vector.max_index(idx[:, 0:8], vals[:, 0:8], tcc[:])
                nc.vector.match_replace(tcc[:], vals[:, 0:8], tcc[:], REPLACED)
                nc.vector.max(vals[:, 8:16], tcc[:])
                nc.vector.max_index(idx[:, 8:16], vals[:, 8:16], tcc[:])

                # --- softmax over the top-15 ---
                negmax = small.tile([128, 1], f32, tag="negmax")
                nc.scalar.mul(negmax[:], vals[:, 0:1], -1.0)
                ex = small.tile([128, TOP_K], f32, tag="ex")
                nc.scalar.activation(
                    ex[:], vals[:, 0:TOP_K], Act.Exp, bias=negmax[:], scale=1.0
                )
                s15 = small.tile([128, 1], f32, tag="s15")
                nc.vector.reduce_sum(s15[:], ex[:], axis=X)
                rcp = small.tile([128, 1], f32, tag="rcp")
                nc.vector.reciprocal(rcp[:], s15[:])
                wts = small.tile([128, TOP_K], f32, tag="wts")
                nc.vector.tensor_scalar_mul(wts[:], ex[:], rcp[:])

                nc.sync.dma_start(
                    out=oidx[t * 128:(t + 1) * 128, :],
                    in_=idx[:, 0:TOP_K].bitcast(dt.int32),
                )
                nc.sync.dma_start(out=owgt[t * 128:(t + 1) * 128, :], in_=wts[:])

    nc.compile()
    _PROGRAM_CACHE["nc"] = nc
    return nc


def _host_prep(embeddings, W, att, corr_lambda, sector_mask, active_mask,
               corr_edge_index, corr_edge_weight):
    """Host-side input restructuring (no score arithmetic)."""
    emb = np.ascontiguousarray(embeddings, dtype=np.float32)
    Wm = np.ascontiguousarray(W, dtype=np.float32)

    # Asp placement matrix: pure re-layout of att (+ fixed leaky-relu consts).
    att = np.asarray(att, dtype=np.float32)
    asp = np.zeros((H, 16), dtype=np.float32)
    for h in range(HEADS):
        rows = slice(64 * h, 64 * (h + 1))
        asp[rows, h] = NEG_SLOPE * att[h, :HD]            # 0.2 * a_dst_h
        asp[rows, 4 + h] = NEG_SLOPE * att[h, HD:]        # 0.2 * a_src_h
        asp[rows, 8] = (NEG_SLOPE / HEADS) * att[h, :HD]  # 0.05 * sum_h a_dst
        asp[rows, 9] = (NEG_SLOPE / HEADS) * att[h, HD:]  # 0.05 * sum_h a_src
    asp = np.ascontiguousarray(asp)

    # combo = where(mask, lambda*corr_dense, -1e30): merge of mask + corr prior.
    active = np.asarray(active_mask, dtype=bool)
    mask = np.asarray(sector_mask, dtype=bool) & active[:, None] & active[None, :]
    corr = np.zeros((N, N), dtype=np.float32)
    ei = np.asarray(corr_edge_index)
    # numpy fancy-assignment: last duplicate wins (matches jax .at[].set on CPU)
    corr[ei[0], ei[1]] = np.asarray(corr_edge_weight, dtype=np.float32)
    lam = np.float32(np.asarray(corr_lambda).reshape(-1)[0])
    combo = np.where(mask, lam * corr, np.float32(MASK_NEG)).astype(np.float32)
    return emb, Wm, asp, combo


def kernel(embeddings, W, att, corr_lambda, sector_mask, active_mask,
           corr_edge_index, corr_edge_weight):
    from concourse.bass_utils import run_bass_kernel_spmd

    emb, Wm, asp, combo = _host_prep(
        embeddings, W, att, corr_lambda, sector_mask, active_mask,
        corr_edge_index, corr_edge_weight)

    nc = _build_program()

    in_maps = []
    for c in range(NCORES):
        in_maps.append({
            "emb": emb,
            "embr": np.ascontiguousarray(emb[c * RPC:(c + 1) * RPC, :]),
            "wmat": Wm,
            "asp": asp,
            "combo": np.ascontiguousarray(combo[c * RPC:(c + 1) * RPC, :]),
        })

    res = run_bass_kernel_spmd(nc, in_maps, core_ids=list(range(NCORES)))

    src = np.concatenate(
        [res.results[c]["oidx"].reshape(-1) for c in range(NCORES)]
    ).astype(np.int32)
    wgt = np.concatenate(
        [res.results[c]["owgt"].reshape(-1) for c in range(NCORES)]
    ).astype(np.float32)
    rows = np.repeat(np.arange(N, dtype=np.int32), TOP_K)
    edge_index = np.stack([src, rows], axis=0)
    return edge_index, wgt
